# revision 16
# baseline (speedup 1.0000x reference)
"""Trainium2 Bass kernel for nn_MultiHeadAttention_77249281786483.

GQA multi-head attention (B=2, S=2048, D=2048, H=32, KVH=8, HD=64) with
interleaved RoPE and causal masking, distributed over 8 NeuronCores with
zero collectives:

  - core c -> batch b=c//4, stripe j=c%4, owning q-tiles {j, 4+j, 8+j, 12+j}
    (4 x 128 query rows).  Striped assignment makes the causal attention
    work identical on every core (per q-tile 4i+j: i+1 key-superblocks of
    512, the last one masked), so a single SPMD graph serves all cores and
    all per-core variation lives in the input data.
  - every core computes the full K/V for its batch; Q only for its rows.
  - bf16 matmuls with fp32 PSUM accumulation; scores are built transposed
    [k, q] so the softmax denominator comes from a ones-augmented V column
    and the context lands directly in the [head-dim, q] layout the output
    projection consumes.
  - RoPE is applied in the transposed projection layout via host-built
    cos/sin tables plus a partition-swap matmul (p <-> p^32) against a
    permutation matrix; Q/K head dims are stored de-interleaved
    (per-64-row head block: 32 even dims then 32 odd dims), which the host
    bakes into the W_q/W_k column order and un-bakes via the W_o row order.
"""

from contextlib import ExitStack

import numpy as np
import ml_dtypes

import concourse.bass as bass  # noqa: F401  (bass types via bacc)
import concourse.mybir as mybir
import concourse.tile as tile
from concourse import bacc
from concourse.bass_utils import run_bass_kernel_spmd

BF16 = mybir.dt.bfloat16
F32 = mybir.dt.float32
BFNP = ml_dtypes.bfloat16

B, S, D = 2, 2048, 2048
H, KVH, HD = 32, 8, 64
N_CORES = 8

MULT = mybir.AluOpType.mult
ADD = mybir.AluOpType.add
EXP = mybir.ActivationFunctionType.Exp

_BUILD_CACHE = {}


def _qhead(m, s):
    # q-head order: chunk m (0..15), slot s (0,1).  Slot parity matches the
    # kv-head parity so GQA score matmuls have equal operand base partitions.
    return 8 * (m // 4) + 4 * s + (m % 4)


def _build_nc(phases="ABC"):
    nc = bacc.Bacc("TRN2", target_bir_lowering=False, debug=False,
                   num_devices=N_CORES)

    xT_d = nc.dram_tensor("xT", [D, S], BF16, kind="ExternalInput").ap()
    xqT_d = nc.dram_tensor("xqT", [D, 512], BF16, kind="ExternalInput").ap()
    wq_d = nc.dram_tensor("wq", [16 * D, 128], BF16, kind="ExternalInput").ap()
    wk_d = nc.dram_tensor("wk", [D, 512], BF16, kind="ExternalInput").ap()
    wv_d = nc.dram_tensor("wv", [D, 512], BF16, kind="ExternalInput").ap()
    wo_d = nc.dram_tensor("wo", [4 * 2048, 512], BF16, kind="ExternalInput").ap()
    bo_d = nc.dram_tensor("bo", [1, D], BF16, kind="ExternalInput").ap()
    cq_d = nc.dram_tensor("cq", [128, 512], F32, kind="ExternalInput").ap()
    sq_d = nc.dram_tensor("sq", [128, 512], F32, kind="ExternalInput").ap()
    ck_d = nc.dram_tensor("ck", [128, 2048], F32, kind="ExternalInput").ap()
    sk_d = nc.dram_tensor("sk", [128, 2048], F32, kind="ExternalInput").ap()
    keep_d = nc.dram_tensor("keep", [128, 1024], BF16,
                            kind="ExternalInput").ap()
    p128_d = nc.dram_tensor("p128", [128, 128], BF16, kind="ExternalInput").ap()
    out_d = nc.dram_tensor("out", [512, D], F32, kind="ExternalOutput").ap()

    with tile.TileContext(nc) as tc:
        with tc.tile_pool(name="pers", bufs=1) as pers:
            qt = pers.tile([128, 16, 512], BF16, tag="qt")
            ktq = pers.tile([128, 4, 2048], BF16, tag="ktq")
            vv = pers.tile([128, 16, 8, 128], BF16, tag="vv")
            ctxT = pers.tile([128, 16, 512], BF16, tag="ctxT")
            keep = pers.tile([128, 4, 2, 128], BF16, tag="keep")
            bo_s = pers.tile([1, D], BF16, tag="bo")
            ones128 = pers.tile([1, 128], BF16, tag="ones128")

            nc.sync.dma_start(
                out=keep[:],
                in_=keep_d.rearrange("p (k s n) -> p k s n", k=4, s=2))
            nc.sync.dma_start(out=bo_s[:], in_=bo_d)
            nc.vector.memset(ones128[:], 1.0)
            nc.vector.memset(vv[:, :, :, 64:128], 1.0)

            # ---------------- Phase A: projections + RoPE ----------------
            with tc.tile_pool(name="pa", bufs=1) as pa, \
                 tc.tile_pool(name="paw", bufs=2) as paw, \
                 tc.tile_pool(name="pscr", bufs=2) as pscr, \
                 tc.tile_pool(name="psA", bufs=4, space="PSUM") as psA:
                # xT held one position-half at a time (SBUF budget)
                xT = pa.tile([128, 16, 1024], BF16, tag="xT")
                xqT = pa.tile([128, 16, 512], BF16, tag="xqT")
                wk = pa.tile([128, 16, 512], BF16, tag="wk")
                wv = pa.tile([128, 16, 512], BF16, tag="wv")
                cqt = pa.tile([128, 512], F32, tag="cqt")
                sqt = pa.tile([128, 512], F32, tag="sqt")
                ckt = pa.tile([128, 2048], F32, tag="ckt")
                skt = pa.tile([128, 2048], F32, tag="skt")
                p128 = pa.tile([128, 128], BF16, tag="p128")

                xT_r = xT_d.rearrange("(c p) m -> p c m", p=128)
                xqT_r = xqT_d.rearrange("(c p) m -> p c m", p=128)
                wk_r = wk_d.rearrange("(c p) n -> p c n", p=128)
                wv_r = wv_d.rearrange("(c p) n -> p c n", p=128)
                # small tensors + xqT on the sync queue so Q-proj (which
                # needs only ~3 MB) starts almost immediately; bulk x/K/V
                # weights stream on the scalar HWDGE queue in parallel.
                nc.sync.dma_start(out=cqt[:], in_=cq_d)
                nc.sync.dma_start(out=sqt[:], in_=sq_d)
                nc.sync.dma_start(out=ckt[:], in_=ck_d)
                nc.sync.dma_start(out=skt[:], in_=sk_d)
                nc.sync.dma_start(out=p128[:], in_=p128_d)
                for c in range(16):
                    nc.sync.dma_start(out=xqT[:, c, :], in_=xqT_r[:, c, :])
                for c in range(16):
                    nc.scalar.dma_start(out=wv[:, c, :], in_=wv_r[:, c, :])
                    nc.scalar.dma_start(out=wk[:, c, :], in_=wk_r[:, c, :])
                    nc.scalar.dma_start(out=xT[:, c, :],
                                        in_=xT_r[:, c, 0:1024])

                # RoPE is software-pipelined: the partition-swap matmul and
                # the rotation TTs for unit i are emitted after unit i+1's
                # projection matmuls, so the PE never waits on the DVE.
                rope_pend = []

                def rope_part1(ps_in, cslice, sslice, out_ap, name):
                    raw = pscr.tile([128, 512], BF16, tag="raw",
                                    name=f"raw{name}")
                    nc.vector.tensor_copy(out=raw[:], in_=ps_in[:])
                    rope_pend.append((ps_in, raw, cslice, sslice, out_ap))

                def rope_part2():
                    ps_in, raw, cslice, sslice, out_ap = rope_pend.pop(0)
                    psw = psA.tile([128, 512], F32, tag="sw", bufs=2)
                    nc.tensor.matmul(psw[:], p128[:], raw[:],
                                     start=True, stop=True)
                    t1 = pscr.tile([128, 512], F32, tag="t1")
                    t2 = pscr.tile([128, 512], F32, tag="t2")
                    nc.vector.tensor_tensor(out=t1[:], in0=ps_in[:],
                                            in1=cslice, op=MULT)
                    nc.vector.tensor_tensor(out=t2[:], in0=psw[:],
                                            in1=sslice, op=MULT)
                    nc.vector.tensor_tensor(out=out_ap, in0=t1[:],
                                            in1=t2[:], op=ADD)

                # Q projection first: xqT + one 0.5 MB weight slice per chunk
                for c in range(16):
                    wqs = paw.tile([128, 16, 128], BF16, tag="wq",
                                   name=f"wqs{c}")
                    wq_slice = wq_d[2048 * c:2048 * (c + 1), :] \
                        .rearrange("(dc p) n -> p dc n", p=128)
                    nc.sync.dma_start(out=wqs[:], in_=wq_slice)
                    ps = psA.tile([128, 512], F32, tag="mm", name=f"psq{c}")
                    for dc in range(16):
                        nc.tensor.matmul(ps[:], wqs[:, dc, :], xqT[:, dc, :],
                                         start=(dc == 0), stop=(dc == 15))
                    rope_part1(ps, cqt[:], sqt[:], qt[:, c, :], f"q{c}")
                    if len(rope_pend) > 1:
                        rope_part2()

                for half in range(2):
                    if half == 1:
                        for c in range(16):
                            nc.scalar.dma_start(
                                out=xT[:, c, :],
                                in_=xT_r[:, c, 1024:2048])
                    # V projection: V[m, n] natural, per kv-head into the
                    # ones-augmented vv layout.
                    for mt in range(8 * half, 8 * (half + 1)):
                        ps = psA.tile([128, 512], F32, tag="mm",
                                      name=f"psv{mt}")
                        mloc = 128 * (mt % 8)
                        for dc in range(16):
                            nc.tensor.matmul(
                                ps[:], xT[:, dc, mloc:mloc + 128],
                                wv[:, dc, :], start=(dc == 0), stop=(dc == 15))
                        if rope_pend:
                            rope_part2()
                        nc.vector.tensor_copy(
                            out=vv[:, mt, :, 0:64],
                            in_=ps[:].rearrange("p (g d) -> p g d", g=8))
                    # K projection (transposed layout) + RoPE
                    for cc in range(4):
                        for ms in range(2 * half, 2 * (half + 1)):
                            ps = psA.tile([128, 512], F32, tag="mm",
                                          name=f"psk{cc}_{ms}")
                            msl = 512 * (ms % 2)
                            for dc in range(16):
                                nc.tensor.matmul(
                                    ps[:], wk[:, dc, 128 * cc:128 * (cc + 1)],
                                    xT[:, dc, msl:msl + 512],
                                    start=(dc == 0), stop=(dc == 15))
                            rope_part1(ps, ckt[:, 512 * ms:512 * (ms + 1)],
                                       skt[:, 512 * ms:512 * (ms + 1)],
                                       ktq[:, cc, 512 * ms:512 * (ms + 1)],
                                       f"k{cc}_{ms}")
                            if len(rope_pend) > 1:
                                rope_part2()
                while rope_pend:
                    rope_part2()

            def dump_and_stop(src_tile, nchunks):
                # debug: dump a persistent bf16 tile to `out` and skip the rest
                with tc.tile_pool(name="dbg", bufs=2) as dbg:
                    for c in range(4):
                        og = dbg.tile([128, 512], F32, tag="og",
                                      name=f"dbg{c}")
                        nc.vector.tensor_copy(
                            out=og[:], in_=src_tile[:, c % nchunks, :])
                        nc.sync.dma_start(
                            out=out_d[128 * c:128 * (c + 1), 0:512],
                            in_=og[:])
                        nc.sync.dma_start(
                            out=out_d[128 * c:128 * (c + 1), 512:1024],
                            in_=og[:])
                        nc.sync.dma_start(
                            out=out_d[128 * c:128 * (c + 1), 1024:1536],
                            in_=og[:])
                        nc.sync.dma_start(
                            out=out_d[128 * c:128 * (c + 1), 1536:2048],
                            in_=og[:])

            if "B" not in phases:
                dump_and_stop(qt, 16)

            # ---------------- Phase B: attention ----------------
            # Software-pipelined: ctx matmuls for block kt are issued LAG
            # iterations after its scores, so the PE never waits inline on
            # the exp/mask chain.  V_aug's 64 ones-columns make the ctx
            # matmul emit the softmax denominator pre-broadcast to 64
            # partitions (rows 64:128 of cx) at no extra PE cost.
            LAG = 2
            if "B" in phases:
              with tc.tile_pool(name="pb", bufs=4) as pb, \
                 tc.tile_pool(name="pbn", bufs=2) as pbn, \
                 tc.tile_pool(name="psS", bufs=2, space="PSUM") as psS, \
                 tc.tile_pool(name="psC", bufs=4, space="PSUM") as psC:
                for m in range(16):
                    kvc = m // 4
                    cxs = [psC.tile([128, 512], F32, tag="ctx",
                                    name=f"cx{m}_{s}")
                           for s in range(2)]
                    ats = {}
                    for step in range(16 + LAG):
                        if step < 16:
                            kt = step
                            sb = kt // 4
                            q0 = 128 * sb
                            nq = 512 - q0
                            sc = psS.tile([128, 1024], F32, tag="sc",
                                          name=f"sc{m}_{kt}")
                            for s in range(2):
                                nc.tensor.matmul(
                                    sc[0:128, 512 * s:512 * s + nq],
                                    ktq[64 * s:64 * (s + 1), kvc,
                                        128 * kt:128 * (kt + 1)],
                                    qt[64 * s:64 * (s + 1), m, q0:512],
                                    start=True, stop=True)
                            at = pb.tile([128, 2, 512], BF16, tag="at",
                                         name=f"at{m}_{kt}")
                            ats[kt] = at
                            scv = sc[:].rearrange("p (s n) -> p s n", s=2)
                            nc.scalar.activation(out=at[:, :, 0:nq],
                                                 in_=scv[:, :, 0:nq],
                                                 func=EXP, scale=0.125)
                            nc.vector.tensor_tensor(
                                out=at[:, :, 0:128], in0=at[:, :, 0:128],
                                in1=keep[:, kt % 4, :, :], op=MULT)
                        if step >= LAG:
                            kt = step - LAG
                            q0 = 128 * (kt // 4)
                            nq = 512 - q0
                            at = ats.pop(kt)
                            for s in range(2):
                                g = 2 * kvc + s
                                nc.tensor.matmul(
                                    cxs[s][0:128, q0:512],
                                    vv[:, kt, g, :],
                                    at[:, s, 0:nq],
                                    start=(kt == 0), stop=(kt == 15))
                    for s in range(2):
                        sums = pbn.tile([64, 512], F32, tag="sums",
                                        name=f"sums{m}_{s}")
                        nc.vector.tensor_copy(out=sums[:],
                                              in_=cxs[s][64:128, :])
                        rec = pbn.tile([64, 512], F32, tag="rec",
                                       name=f"rec{m}_{s}")
                        scr2 = pbn.tile([64, 512], F32, tag="scr2",
                                        name=f"scr2{m}_{s}")
                        nc.vector.reciprocal_approx_accurate(
                            out=rec[:], in_=sums[:], scratch=scr2[:])
                        nc.vector.tensor_tensor(
                            out=ctxT[64 * s:64 * (s + 1), m, :],
                            in0=cxs[s][0:64, :], in1=rec[:], op=MULT)

            if "C" not in phases and "B" in phases:
                dump_and_stop(ctxT, 16)

            # ---------------- Phase C: output projection ----------------
            if "C" in phases:
              with tc.tile_pool(name="pcw", bufs=2) as pcw, \
                 tc.tile_pool(name="pco", bufs=2) as pco, \
                 tc.tile_pool(name="psO", bufs=4, space="PSUM") as psO:
                for es in range(4):
                    woe = pcw.tile([128, 16, 512], BF16, tag="wo")
                    wo_slice = wo_d[2048 * es:2048 * (es + 1), :] \
                        .rearrange("(c p) n -> p c n", p=128)
                    nc.sync.dma_start(out=woe[:], in_=wo_slice)
                    for qi in range(4):
                        po = psO.tile([128, 512], F32, tag="out")
                        nc.tensor.matmul(po[:], ones128[:],
                                         bo_s[:, 512 * es:512 * (es + 1)],
                                         tile_position=(0, 0),
                                         start=True, stop=False)
                        for c in range(16):
                            nc.tensor.matmul(
                                po[:], ctxT[:, c, 128 * qi:128 * (qi + 1)],
                                woe[:, c, :],
                                start=False, stop=(c == 15))
                        og = pco.tile([128, 512], F32, tag="og")
                        nc.vector.tensor_copy(out=og[:], in_=po[:])
                        nc.sync.dma_start(
                            out=out_d[128 * qi:128 * (qi + 1),
                                      512 * es:512 * (es + 1)],
                            in_=og[:])

    nc.compile()
    return nc


def _build_nc_v3():
    nc = bacc.Bacc("TRN2", target_bir_lowering=False, debug=False,
                   num_devices=N_CORES)

    xT_d = nc.dram_tensor("xT", [D, S], BF16, kind="ExternalInput").ap()
    xqT_d = nc.dram_tensor("xqT", [D, 512], BF16, kind="ExternalInput").ap()
    wq_d = nc.dram_tensor("wq", [16 * D, 128], BF16, kind="ExternalInput").ap()
    wk_d = nc.dram_tensor("wk", [D, 512], BF16, kind="ExternalInput").ap()
    wv_d = nc.dram_tensor("wv", [D, 512], BF16, kind="ExternalInput").ap()
    wo_d = nc.dram_tensor("wo", [4 * 2048, 512], BF16,
                          kind="ExternalInput").ap()
    bo_d = nc.dram_tensor("bo", [1, D], BF16, kind="ExternalInput").ap()
    cq_d = nc.dram_tensor("cq", [128, 512], F32, kind="ExternalInput").ap()
    sq_d = nc.dram_tensor("sq", [128, 512], F32, kind="ExternalInput").ap()
    ck_d = nc.dram_tensor("ck", [128, 2048], F32, kind="ExternalInput").ap()
    sk_d = nc.dram_tensor("sk", [128, 2048], F32, kind="ExternalInput").ap()
    keep_d = nc.dram_tensor("keep", [128, 1024], BF16,
                            kind="ExternalInput").ap()
    p128_d = nc.dram_tensor("p128", [128, 128], BF16,
                            kind="ExternalInput").ap()
    out_d = nc.dram_tensor("out", [512, D], F32, kind="ExternalOutput").ap()

    Q_PREFIX = 8   # Q chunks built before V/K; the rest interleave with pairs
    LAG = 3        # attention ctx-matmul software pipeline depth

    with ExitStack() as st:
        tc = st.enter_context(tile.TileContext(nc))
        pers = st.enter_context(tc.tile_pool(name="pers", bufs=1))
        qt = pers.tile([128, 16, 512], BF16, tag="qt")
        ktq = pers.tile([128, 4, 2048], BF16, tag="ktq")
        vv = pers.tile([128, 16, 8, 128], BF16, tag="vv")
        ctxT = pers.tile([128, 16, 512], BF16, tag="ctxT")
        keep = pers.tile([128, 4, 2, 128], BF16, tag="keep")
        bo_s = pers.tile([1, D], BF16, tag="bo")
        ones128 = pers.tile([1, 128], BF16, tag="ones128")

        nc.vector.memset(ones128[:], 1.0)
        nc.vector.memset(vv[:, :, :, 64:128], 1.0)

        # Q-side pools live from the prefix through the merged section.
        paQ = st.enter_context(tc.tile_pool(name="paQ", bufs=1))
        paw = st.enter_context(tc.tile_pool(name="paw", bufs=2))
        pscr = st.enter_context(tc.tile_pool(name="pscr", bufs=2))
        psQ2 = st.enter_context(tc.tile_pool(name="psQ2", bufs=1,
                                             space="PSUM"))
        xqT = paQ.tile([128, 16, 512], BF16, tag="xqT")
        cqt = paQ.tile([128, 512], F32, tag="cqt")
        sqt = paQ.tile([128, 512], F32, tag="sqt")
        p128 = paQ.tile([128, 128], BF16, tag="p128")

        xT_r = xT_d.rearrange("(c p) m -> p c m", p=128)
        xqT_r = xqT_d.rearrange("(c p) m -> p c m", p=128)
        wk_r = wk_d.rearrange("(c p) n -> p c n", p=128)
        wv_r = wv_d.rearrange("(c p) n -> p c n", p=128)

        for c in range(16):
            nc.sync.dma_start(out=xqT[:, c, :], in_=xqT_r[:, c, :])
        nc.sync.dma_start(out=p128[:], in_=p128_d)
        nc.sync.dma_start(out=cqt[:], in_=cq_d)
        nc.sync.dma_start(out=sqt[:], in_=sq_d)
        nc.sync.dma_start(
            out=keep[:], in_=keep_d.rearrange("p (k s n) -> p k s n",
                                              k=4, s=2))
        nc.sync.dma_start(out=bo_s[:], in_=bo_d)

        def q_start(c):
            wqs = paw.tile([128, 16, 128], BF16, tag="wq", name=f"wqs{c}")
            wq_slice = wq_d[2048 * c:2048 * (c + 1), :] \
                .rearrange("(dc p) n -> p dc n", p=128)
            nc.sync.dma_start(out=wqs[:], in_=wq_slice)
            ps = psQ2.tile([128, 512], F32, tag="qmm", name=f"psq{c}",
                           bufs=1)
            return wqs, ps

        def q_mm(st_, c, dc):
            wqs, ps = st_
            nc.tensor.matmul(ps[:], wqs[:, dc, :], xqT[:, dc, :],
                             start=(dc == 0), stop=(dc == 15))

        def q_rope(st_, c, swpool, swtag, swshape):
            wqs, ps = st_
            raw = pscr.tile([128, 512], BF16, tag="raw", name=f"rawq{c}")
            nc.vector.tensor_copy(out=raw[:], in_=ps[:])
            psw = swpool.tile(swshape, F32, tag=swtag, name=f"pswq{c}",
                              bufs=2)
            nc.tensor.matmul(psw[0:128, 0:512], p128[:], raw[:],
                             start=True, stop=True)
            t1 = pscr.tile([128, 512], F32, tag="t1", name=f"t1q{c}")
            t2 = pscr.tile([128, 512], F32, tag="t2", name=f"t2q{c}")
            nc.vector.tensor_tensor(out=t1[:], in0=ps[:], in1=cqt[:], op=MULT)
            nc.vector.tensor_tensor(out=t2[:], in0=psw[0:128, 0:512],
                                    in1=sqt[:], op=MULT)
            nc.vector.tensor_tensor(out=qt[:, c, :], in0=t1[:], in1=t2[:],
                                    op=ADD)

        def q_rope_dve(st_, c):
            # PE-free RoPE: bf16 staging + 4 cross-quadrant DVE copies
            # implement the p <-> p^32 partition swap.
            wqs, ps = st_
            raw = pscr.tile([128, 512], BF16, tag="raw", name=f"rawq{c}")
            nc.vector.tensor_copy(out=raw[:], in_=ps[:])
            qsw = pscr.tile([128, 512], BF16, tag="qswp", name=f"qswp{c}")
            for blk in range(4):
                srcb = (blk ^ 1) * 32
                nc.vector.tensor_copy(out=qsw[32 * blk:32 * blk + 32, :],
                                      in_=raw[srcb:srcb + 32, :])
            t1 = pscr.tile([128, 512], F32, tag="t1", name=f"t1q{c}")
            t2 = pscr.tile([128, 512], F32, tag="t2", name=f"t2q{c}")
            nc.vector.tensor_tensor(out=t1[:], in0=raw[:], in1=cqt[:],
                                    op=MULT)
            nc.vector.tensor_tensor(out=t2[:], in0=qsw[:], in1=sqt[:],
                                    op=MULT)
            nc.vector.tensor_tensor(out=qt[:, c, :], in0=t1[:], in1=t2[:],
                                    op=ADD)

        def q_proj(c, swpool):
            st_ = q_start(c)
            for dc in range(16):
                q_mm(st_, c, dc)
            q_rope(st_, c, swpool, "sw", [128, 512])

        # ---- V/K section (own pools, closed afterwards) ----
        with tc.tile_pool(name="paVK", bufs=1) as paVK, \
             tc.tile_pool(name="psVK", bufs=4, space="PSUM") as psVK:
            xT = paVK.tile([128, 16, 1024], BF16, tag="xT")
            wk = paVK.tile([128, 16, 512], BF16, tag="wk")
            wv = paVK.tile([128, 16, 512], BF16, tag="wv")
            ckt = paVK.tile([128, 2048], F32, tag="ckt")
            skt = paVK.tile([128, 2048], F32, tag="skt")
            # Q prefix first: its (small) inputs head the queue, the
            # bulk x/K/V stream is queued behind and overlaps the compute.
            for c in range(Q_PREFIX):
                q_proj(c, psVK)
            for c in range(16):
                nc.sync.dma_start(out=wv[:, c, :], in_=wv_r[:, c, :])
                nc.sync.dma_start(out=wk[:, c, :], in_=wk_r[:, c, :])
            for c in range(16):
                nc.sync.dma_start(out=xT[:, c, :], in_=xT_r[:, c, 0:1024])
            nc.sync.dma_start(out=ckt[:], in_=ck_d)
            nc.sync.dma_start(out=skt[:], in_=sk_d)

            rope_pend = []

            def rope_part1(ps_in, cslice, sslice, out_ap, name):
                raw = pscr.tile([128, 512], BF16, tag="raw",
                                name=f"raw{name}")
                nc.vector.tensor_copy(out=raw[:], in_=ps_in[:])
                rope_pend.append((ps_in, raw, cslice, sslice, out_ap))

            def rope_part2():
                ps_in, raw, cslice, sslice, out_ap = rope_pend.pop(0)
                psw = psVK.tile([128, 512], F32, tag="sw", bufs=2)
                nc.tensor.matmul(psw[:], p128[:], raw[:],
                                 start=True, stop=True)
                t1 = pscr.tile([128, 512], F32, tag="t1")
                t2 = pscr.tile([128, 512], F32, tag="t2")
                nc.vector.tensor_tensor(out=t1[:], in0=ps_in[:], in1=cslice,
                                        op=MULT)
                nc.vector.tensor_tensor(out=t2[:], in0=psw[:], in1=sslice,
                                        op=MULT)
                nc.vector.tensor_tensor(out=out_ap, in0=t1[:], in1=t2[:],
                                        op=ADD)

            for half in range(2):
                if half == 1:
                    for c in range(16):
                        nc.scalar.dma_start(out=xT[:, c, :],
                                            in_=xT_r[:, c, 1024:2048])
                for mt in range(8 * half, 8 * (half + 1)):
                    ps = psVK.tile([128, 512], F32, tag="mm", name=f"psv{mt}")
                    mloc = 128 * (mt % 8)
                    for dc in range(16):
                        nc.tensor.matmul(
                            ps[:], xT[:, dc, mloc:mloc + 128], wv[:, dc, :],
                            start=(dc == 0), stop=(dc == 15))
                    if rope_pend:
                        rope_part2()
                    nc.vector.tensor_copy(
                        out=vv[:, mt, :, 0:64],
                        in_=ps[:].rearrange("p (g d) -> p g d", g=8))
                for cc in range(4):
                    for ms in range(2 * half, 2 * (half + 1)):
                        ps = psVK.tile([128, 512], F32, tag="mm",
                                       name=f"psk{cc}_{ms}")
                        msl = 512 * (ms % 2)
                        for dc in range(16):
                            nc.tensor.matmul(
                                ps[:], wk[:, dc, 128 * cc:128 * (cc + 1)],
                                xT[:, dc, msl:msl + 512],
                                start=(dc == 0), stop=(dc == 15))
                        rope_part1(ps, ckt[:, 512 * ms:512 * (ms + 1)],
                                   skt[:, 512 * ms:512 * (ms + 1)],
                                   ktq[:, cc, 512 * ms:512 * (ms + 1)],
                                   f"k{cc}_{ms}")
                        if len(rope_pend) > 1:
                            rope_part2()
            while rope_pend:
                rope_part2()

        # ---- merged section: attention pairs + remaining Q chunks ----
        pcw = st.enter_context(tc.tile_pool(name="pcw", bufs=4))
        stq = ExitStack()
        pb = stq.enter_context(tc.tile_pool(name="pb", bufs=6))
        pbn = stq.enter_context(tc.tile_pool(name="pbn", bufs=2))
        psS = stq.enter_context(tc.tile_pool(name="psS", bufs=2,
                                             space="PSUM"))
        psC = stq.enter_context(tc.tile_pool(name="psC", bufs=3,
                                             space="PSUM"))

        deferred = []

        def attention_pair(m, qc=None):
            if qc is not None:
                qst = q_start(qc)
            kvc = m // 4
            cxs = [psC.tile([128, 512], F32, tag="ctx", name=f"cx{m}_{s}")
                   for s in range(2)]
            ats = {}
            for step in range(16 + LAG):
                if step < 16:
                    kt = step
                    q0 = 128 * (kt // 4)
                    nq = 512 - q0
                    sc = psS.tile([128, 1024], F32, tag="sc",
                                  name=f"sc{m}_{kt}")
                    for s in range(2):
                        nc.tensor.matmul(
                            sc[0:128, 512 * s:512 * s + nq],
                            ktq[64 * s:64 * (s + 1), kvc,
                                128 * kt:128 * (kt + 1)],
                            qt[64 * s:64 * (s + 1), m, q0:512],
                            start=True, stop=True)
                    at = pb.tile([128, 2, 512], BF16, tag="at",
                                 name=f"at{m}_{kt}")
                    ats[kt] = at
                    scv = sc[:].rearrange("p (s n) -> p s n", s=2)
                    nc.scalar.activation(out=at[:, :, 0:nq],
                                         in_=scv[:, :, 0:nq],
                                         func=EXP, scale=0.125)
                    nc.vector.tensor_tensor(
                        out=at[:, :, 0:128], in0=at[:, :, 0:128],
                        in1=keep[:, kt % 4, :, :], op=MULT)
                    if qc is not None:
                        q_mm(qst, qc, kt)
                    if deferred:
                        deferred.pop(0)()
                if step >= LAG:
                    kt = step - LAG
                    q0 = 128 * (kt // 4)
                    nq = 512 - q0
                    at = ats.pop(kt)
                    for s in range(2):
                        nc.tensor.matmul(
                            cxs[s][0:128, q0:512],
                            vv[:, kt, 2 * kvc + s, :],
                            at[:, s, 0:nq],
                            start=(kt == 0), stop=(kt == 15))
            if qc is not None:
                raw = pscr.tile([128, 512], BF16, tag="raw",
                                name=f"rawq{qc}")
                nc.vector.tensor_copy(out=raw[:], in_=qst[1][:])

                def rope_rest(c=qc, raw=raw):
                    qsw = pscr.tile([128, 512], BF16, tag="qswp",
                                    name=f"qswp{c}")
                    for blk in range(4):
                        srcb = (blk ^ 1) * 32
                        nc.vector.tensor_copy(
                            out=qsw[32 * blk:32 * blk + 32, :],
                            in_=raw[srcb:srcb + 32, :])
                    t1 = pscr.tile([128, 512], F32, tag="t1", name=f"t1q{c}")
                    t2 = pscr.tile([128, 512], F32, tag="t2", name=f"t2q{c}")
                    nc.vector.tensor_tensor(out=t1[:], in0=raw[:],
                                            in1=cqt[:], op=MULT)
                    nc.vector.tensor_tensor(out=t2[:], in0=qsw[:],
                                            in1=sqt[:], op=MULT)
                    nc.vector.tensor_tensor(out=qt[:, c, :], in0=t1[:],
                                            in1=t2[:], op=ADD)

                deferred.append(rope_rest)

            def norm(s, m=m, cxs=cxs):
                sums = pbn.tile([64, 512], F32, tag="sums",
                                name=f"sums{m}_{s}")
                nc.vector.tensor_copy(out=sums[:], in_=cxs[s][64:128, :])
                rec = pbn.tile([64, 512], F32, tag="rec", name=f"rec{m}_{s}")
                scr2 = pbn.tile([64, 512], F32, tag="scr2",
                                name=f"scr2{m}_{s}")
                nc.vector.reciprocal_approx_accurate(
                    out=rec[:], in_=sums[:], scratch=scr2[:])
                nc.vector.tensor_tensor(
                    out=ctxT[64 * s:64 * (s + 1), m, :],
                    in0=cxs[s][0:64, :], in1=rec[:], op=MULT)

            deferred.append(lambda: norm(0))
            deferred.append(lambda: norm(1))

        woq = {}

        def woe_dma(es, cq):
            w = pcw.tile([128, 4, 512], BF16, tag="wo",
                         name=f"wo{es}_{cq}")
            base = 2048 * es + 512 * cq
            nc.sync.dma_start(
                out=w[:],
                in_=wo_d[base:base + 512, :].rearrange("(c p) n -> p c n",
                                                       p=128))
            woq[(es, cq)] = w

        for i in range(16):
            c = i + Q_PREFIX
            attention_pair(i, qc=c if c < 16 else None)
        while deferred:
            deferred.pop(0)()
        woe_dma(0, 0)
        woe_dma(0, 1)
        stq.close()  # release merged-section PSUM/SBUF pools

        # ---- output projection ----
        pco = st.enter_context(tc.tile_pool(name="pco", bufs=2))
        psO = st.enter_context(tc.tile_pool(name="psO", bufs=4, space="PSUM"))
        for es in range(4):
            pos = [psO.tile([128, 512], F32, tag="out", name=f"po{es}_{qi}")
                   for qi in range(4)]
            for qi in range(4):
                nc.tensor.matmul(pos[qi][:], ones128[:],
                                 bo_s[:, 512 * es:512 * (es + 1)],
                                 tile_position=(0, 0), start=True, stop=False)
            for cq in range(4):
                if (es, cq) not in woq:
                    woe_dma(es, cq)
                w = woq.pop((es, cq))
                # keep two quarters prefetched ahead
                t = 4 * es + cq
                for t2 in (t + 1, t + 2):
                    if t2 < 16 and (t2 // 4, t2 % 4) not in woq:
                        woe_dma(t2 // 4, t2 % 4)
                for qi in range(4):
                    for c in range(4 * cq, 4 * cq + 4):
                        nc.tensor.matmul(
                            pos[qi][:], ctxT[:, c, 128 * qi:128 * (qi + 1)],
                            w[:, c % 4, :],
                            start=False, stop=(c == 15))
            for qi in range(4):
                og = pco.tile([128, 512], F32, tag="og", name=f"og{es}_{qi}")
                nc.vector.tensor_copy(out=og[:], in_=pos[qi][:])
                nc.scalar.dma_start(
                    out=out_d[128 * qi:128 * (qi + 1),
                              512 * es:512 * (es + 1)],
                    in_=og[:])

    nc.compile()
    return nc


def _get_nc():
    if "nc" not in _BUILD_CACHE:
        _BUILD_CACHE["nc"] = _build_nc_v3()
    return _BUILD_CACHE["nc"]


def _build_perms():
    r = np.arange(2048)
    m, rr = r // 128, r % 128
    s, half, jd = rr // 64, (rr % 64) // 32, rr % 32
    h = 8 * (m // 4) + 4 * s + (m % 4)
    qperm = 64 * h + 2 * jd + half
    woperm = 64 * h + (rr % 64)
    rk = np.arange(512)
    ck, rrk = rk // 128, rk % 128
    sk_, halfk, jdk = rrk // 64, (rrk % 64) // 32, rrk % 32
    kperm = 64 * (2 * ck + sk_) + 2 * jdk + halfk
    return qperm, kperm, woperm


def _rope_tables(fc, fs, positions):
    p = np.arange(128)
    jd = p % 32
    sign = np.where((p % 64) < 32, -1.0, 1.0).astype(np.float32)
    C2 = np.ascontiguousarray(fc[positions][:, jd].T.astype(np.float32))
    S2 = np.ascontiguousarray(
        (fs[positions][:, jd].T * sign[:, None]).astype(np.float32))
    return C2, S2


def prepare_in_maps(inputs):
    x = np.asarray(inputs["x"], np.float32)
    Wq = np.asarray(inputs["W_q"], np.float32)
    Wk = np.asarray(inputs["W_k"], np.float32)
    Wv = np.asarray(inputs["W_v"], np.float32)
    Wo = np.asarray(inputs["W_o"], np.float32)
    bo = np.asarray(inputs["b_o"], np.float32)
    fc = np.asarray(inputs["freqs_cos"], np.float32)
    fs = np.asarray(inputs["freqs_sin"], np.float32)

    qperm, kperm, woperm = _build_perms()
    wq_p = Wq[:, qperm].astype(BFNP)
    # [2048 d, 2048 n] -> n-slices of 128: [16*2048, 128]
    wq_host = np.ascontiguousarray(
        wq_p.reshape(2048, 16, 128).transpose(1, 0, 2)).reshape(16 * 2048, 128)
    wk_host = np.ascontiguousarray(Wk[:, kperm].astype(BFNP))
    wv_host = np.ascontiguousarray(Wv.astype(BFNP))
    wo_p = Wo[woperm, :].astype(BFNP)
    # [2048 dh, 2048 e] -> [es, chunk, 128, 512] -> [4*2048, 512]
    wo_host = np.ascontiguousarray(
        wo_p.reshape(16, 128, 4, 512).transpose(2, 0, 1, 3)).reshape(8192, 512)
    bo_host = np.ascontiguousarray(bo.reshape(1, 2048).astype(BFNP))
    p128 = np.zeros((128, 128), BFNP)
    p128[np.arange(128), np.arange(128) ^ 32] = 1.0
    ck_t, sk_t = _rope_tables(fc, fs, np.arange(2048))

    in_maps = []
    for core in range(N_CORES):
        b, j = core // 4, core % 4
        qpos = (np.arange(4)[:, None] * 512 + 128 * j
                + np.arange(128)[None, :]).reshape(-1)
        xb = x[b]
        xT = np.ascontiguousarray(xb.T.astype(BFNP))
        xqT = np.ascontiguousarray(xb[qpos].T.astype(BFNP))
        cq_t, sq_t = _rope_tables(fc, fs, qpos)
        kp = np.arange(128)[:, None]
        qf = np.arange(128)[None, :]
        keep4 = np.stack(
            [((128 * ktp + kp - 128 * j) <= qf) for ktp in range(4)],
            axis=1)  # [128, 4, 128]
        keep = np.repeat(keep4[:, :, None, :], 2, axis=2) \
            .reshape(128, 1024).astype(BFNP)
        in_maps.append({
            "xT": xT, "xqT": xqT, "wq": wq_host, "wk": wk_host,
            "wv": wv_host, "wo": wo_host, "bo": bo_host,
            "cq": cq_t, "sq": sq_t, "ck": ck_t, "sk": sk_t,
            "keep": np.ascontiguousarray(keep), "p128": p128,
        })
    return in_maps


def kernel(**inputs):
    nc = _get_nc()
    in_maps = prepare_in_maps(inputs)
    res = run_bass_kernel_spmd(nc, in_maps, core_ids=list(range(N_CORES)))
    out = np.zeros((B, S, D), np.float32)
    for core in range(N_CORES):
        b, j = core // 4, core % 4
        qpos = (np.arange(4)[:, None] * 512 + 128 * j
                + np.arange(128)[None, :]).reshape(-1)
        out[b][qpos] = res.results[core]["out"]
    return out
